# revision 13
# baseline (speedup 1.0000x reference)
"""DynamicVoxelizer Bass/Trainium2 kernel.

Contract: kernel(**inputs) takes the FULL inputs (points: [8, 1_000_000, 3]
float32), distributes across 8 NeuronCores (one batch per core), runs a Bass
kernel per core, and returns the FULL outputs matching reference():
  (out_points f32 [8,1M,3], coords_zyx i32 [8,1M,3], point_idxes i32 [8,1M],
   point_offsets f32 [8,1M,3], valid bool [8,1M])

Numerics: XLA compiles the reference's (p - min) / 0.2 into (p - min) * 5.0
(verified bit-exact on both the CPU and neuron backends), so the device
computes q = RN(5 * RN(p - min)) and floor(q) via an RNE cast + compare
fix-up (the hardware f32->i32 conversion rounds to nearest-even).
"""

import numpy as np

import concourse.bass as bass
import concourse.mybir as mybir
import concourse.tile as tile
from concourse.bass_utils import run_bass_kernel_spmd

AOT = mybir.AluOpType
AFT = mybir.ActivationFunctionType
F32 = mybir.dt.float32
I32 = mybir.dt.int32
I16 = mybir.dt.int16
U8 = mybir.dt.uint8

# Problem constants (hardcoded; kernel.py must be self-contained)
B = 8                      # batch == number of cores
NPTS = 1_000_000           # points per batch
P = 128                    # SBUF partitions
NPER = 7840                # points per partition (padded): 128*7840 = 1_003_520
NPAD = P * NPER            # padded points per core
FPER = NPER * 3            # floats per partition = 23520
CH_N = 1120                # points per partition per chunk
CH_F = CH_N * 3            # 3360 floats
NCHUNKS = NPER // CH_N     # 7
PAD_VAL = 1.0e9            # pad points are far out of range -> invalid

MINS = (-51.2, -51.2, -3.0)
GRIDF = (512.0, 512.0, 30.0)
# largest fp32 strictly below the grid bound: q < g  <=>  q <= gprime
GPRIME = tuple(float(np.nextafter(np.float32(g), np.float32(0))) for g in GRIDF)

_cached = {}


def _split_excess_waits(nc, limit=1):
    """walrus on this container rejects instructions with more than `limit`
    sync-wait conditions; split the excess into standalone event-sem waits."""
    for fn in nc.m.functions:
        for bb in fn.blocks:
            il = bb.instructions
            new = []
            for inst in il:
                si = getattr(inst, "sync_info", None)
                if si is not None and si.on_wait and len(si.on_wait) > limit:
                    waits = list(si.on_wait)
                    excess, keep = waits[:-limit], waits[-limit:]
                    for j, w in enumerate(excess):
                        ev = mybir.InstEventSemaphore(
                            name=f"{inst.name}-xw{j}", ins=[], outs=[],
                            sync_info=mybir.SyncInfo(on_wait=[w], on_update=[]),
                        )
                        ev.engine = inst.engine
                        new.append(ev)
                    inst.sync_info = mybir.SyncInfo(
                        on_wait=keep, on_update=list(si.on_update))
                new.append(inst)
            il[:] = new


def _build_kernel():
    nc = bass.Bass()
    # const APs for activation biases (Abs/Sign need AP biases)
    for v in sorted({-g for g in GPRIME}):
        t = nc.alloc_sbuf_tensor(f"const-f32-{v}", [P, 1], F32)
        nc.gpsimd.memset(t.ap(), v)
        nc.const_aps.aps[(F32, v)] = t.ap()
    nc.all_engine_barrier()

    pts = nc.dram_tensor("pts", [P, FPER], F32, kind="ExternalInput")
    w_i = nc.dram_tensor("w_i", [P, P], F32, kind="ExternalInput")
    w_im = nc.dram_tensor("w_im", [P, P], F32, kind="ExternalInput")
    w_p2 = nc.dram_tensor("w_p2", [P, P], F32, kind="ExternalInput")
    w_m2 = nc.dram_tensor("w_m2", [P, P], F32, kind="ExternalInput")
    o_pts = nc.dram_tensor("o_pts", [P, FPER], F32, kind="ExternalOutput")
    o_coords = nc.dram_tensor("o_coords", [P, FPER], I16, kind="ExternalOutput")
    o_off = nc.dram_tensor("o_off", [P, FPER], F32, kind="ExternalOutput")
    o_idx = nc.dram_tensor("o_idx", [P, NPER], I32, kind="ExternalOutput")
    o_valid = nc.dram_tensor("o_valid", [P, NPER], U8, kind="ExternalOutput")

    with tile.TileContext(nc) as tc:
        with tc.tile_pool(name="w", bufs=1) as wp, \
             tc.tile_pool(name="io", bufs=2) as iop, \
             tc.tile_pool(name="scr", bufs=1) as scr, \
             tc.tile_pool(name="ps", bufs=1, space="PSUM") as psp:
            wi = wp.tile([P, P], F32, tag="wi")
            wim = wp.tile([P, P], F32, tag="wim")
            wp2 = wp.tile([P, P], F32, tag="wp2")
            wm2 = wp.tile([P, P], F32, tag="wm2")
            nc.sync.dma_start(wi[:], w_i[:])
            nc.sync.dma_start(wim[:], w_im[:])
            nc.sync.dma_start(wp2[:], w_p2[:])
            nc.sync.dma_start(wm2[:], w_m2[:])
            for ch in range(NCHUNKS):
                f0_ = ch * CH_F     # float column offset
                n0_ = ch * CH_N     # point column offset

                pt = iop.tile([P, CH_F], F32, tag="pts")
                nc.sync.dma_start(pt[:], pts[:, f0_:f0_ + CH_F])
                pt3 = pt[:].rearrange("p (n c) -> p n c", c=3)

                # t3 = p - min (per component, strided)  [ACT]
                t3 = scr.tile([P, CH_F], F32, tag="t3")
                t33 = t3[:].rearrange("p (n c) -> p n c", c=3)
                for c in range(3):
                    nc.scalar.activation(t33[:, :, c], pt3[:, :, c],
                                         AFT.Copy, bias=-MINS[c], scale=1.0)
                # q0 = RN(5*t)  [DVE ts 2x]
                q0 = scr.tile([P, CH_F], F32, tag="q0")
                nc.vector.tensor_scalar(q0[:], t3[:], 5.0, None, AOT.mult)

                # floor(q0): RNE cast + fixup   [ACT casts, DVE fix]
                i0 = scr.tile([P, CH_F], I32, tag="C")
                nc.scalar.copy(i0[:], q0[:])
                flp = scr.tile([P, CH_F], F32, tag="A")   # rne(q0) + 1
                nc.scalar.activation(flp[:], i0[:], AFT.Copy,
                                     bias=1.0, scale=1.0)
                h = scr.tile([P, CH_F], F32, tag="D")
                nc.vector.scalar_tensor_tensor(
                    h[:], q0[:], 1.0, flp[:], AOT.add, AOT.is_lt)
                # cfp1 = floor(q0) + 1 = flp - h   [PE identity matmuls]
                cfp1 = psp.tile([P, CH_F], F32, tag="psA")
                for s in range(0, CH_F, 512):
                    e = min(s + 512, CH_F)
                    nc.tensor.matmul(cfp1[:, s:e], wi[:], flp[:, s:e],
                                     start=True, stop=False)
                    nc.tensor.matmul(cfp1[:, s:e], wim[:], h[:, s:e],
                                     start=False, stop=True)

                # validity: w = |2*q0 - g'|; valid_c <=> w <= g'
                # sign trick: sg = Sign(w - g') ; valid <=> max(sg) <= 0
                w3 = scr.tile([P, CH_F], F32, tag="E")
                w33 = w3[:].rearrange("p (n c) -> p n c", c=3)
                q03 = q0[:].rearrange("p (n c) -> p n c", c=3)
                for c in range(3):
                    nc.scalar.activation(w33[:, :, c], q03[:, :, c],
                                         AFT.Abs, bias=-GPRIME[c], scale=2.0)
                sg = w3
                sg3 = w33
                for c in range(3):
                    nc.scalar.activation(sg3[:, :, c], w33[:, :, c],
                                         AFT.Sign, bias=-GPRIME[c], scale=1.0)
                v01 = scr.tile([P, CH_N], F32, tag="v01")
                nc.vector.tensor_tensor(v01[:], sg3[:, :, 0], sg3[:, :, 1], AOT.max)
                mx = scr.tile([P, CH_N], F32, tag="mx")
                nc.vector.tensor_tensor(mx[:], v01[:], sg3[:, :, 2], AOT.max)
                valid = iop.tile([P, CH_N], U8, tag="valid")
                nc.vector.tensor_scalar(valid[:], mx[:], 0.0, None, AOT.is_le)

                # materialize interleaved f32 mask vm3 (unit-stride users)
                vm3 = scr.tile([P, CH_F], F32, tag="vm3")
                vm33 = vm3[:].rearrange("p (n c) -> p n c", c=3)
                for c in range(3):
                    nc.vector.tensor_scalar(vm33[:, :, c], valid[:], 0.0,
                                            None, AOT.add)

                # coords+1 = valid * (floor+1), i16 out; host subtracts 1
                # and reverses xyz->zyx. Invalid lanes -> 0 -> host -1.
                cmem = iop.tile([P, CH_F], I16, tag="cmem")
                nc.vector.tensor_tensor(cmem[:], cfp1[:], vm3[:], AOT.mult)


                # offsets: off = ((cf*-0.2 + t3) - 0.1) * valid   [DVE]
                off_ps = psp.tile([P, CH_F], F32, tag="psA")
                for s in range(0, CH_F, 512):
                    e = min(s + 512, CH_F)
                    nc.tensor.matmul(off_ps[:, s:e], wi[:], t3[:, s:e],
                                     start=True, stop=False)
                    nc.tensor.matmul(off_ps[:, s:e], wm2[:], flp[:, s:e],
                                     start=False, stop=False)
                    nc.tensor.matmul(off_ps[:, s:e], wp2[:], h[:, s:e],
                                     start=False, stop=True)
                offm = iop.tile([P, CH_F], F32, tag="offm")
                nc.vector.scalar_tensor_tensor(
                    offm[:], off_ps[:], 0.1, vm3[:], AOT.add, AOT.mult)

                # out_points: p * valid  [GP, unit-stride in-place]
                nc.gpsimd.tensor_tensor(pt[:], pt[:], vm3[:], AOT.mult)

                # point idxes + 1 = valid * (iota+1); host subtracts 1
                iot = scr.tile([P, CH_N], I32, tag="iot")
                nc.gpsimd.iota(iot[:], [[1, CH_N]], base=n0_ + 1,
                               channel_multiplier=NPER)
                imem = iop.tile([P, CH_N], I32, tag="imem")
                nc.vector.tensor_tensor(imem[:], iot[:], valid[:], AOT.mult)

                # stores
                nc.sync.dma_start(o_pts[:, f0_:f0_ + CH_F], pt[:])
                nc.sync.dma_start(o_coords[:, f0_:f0_ + CH_F], cmem[:])
                nc.sync.dma_start(o_off[:, f0_:f0_ + CH_F], offm[:])
                nc.sync.dma_start(o_idx[:, n0_:n0_ + CH_N], imem[:])
                nc.sync.dma_start(o_valid[:, n0_:n0_ + CH_N], valid[:])

    _split_excess_waits(nc, limit=1)
    return nc


def _get_nc():
    if "nc" not in _cached:
        _cached["nc"] = _build_kernel()
    return _cached["nc"]


_eye = np.eye(P, dtype=np.float32)
_W = {"i": _eye, "im": (-_eye).astype(np.float32),
      "p2": (np.float32(0.2) * _eye).astype(np.float32),
      "m2": (np.float32(-0.2) * _eye).astype(np.float32)}


def _make_in_maps(points):
    in_maps = []
    for b in range(B):
        flat = np.full(NPAD * 3, PAD_VAL, dtype=np.float32)
        flat[: NPTS * 3] = points[b].reshape(-1)
        in_maps.append({"pts": flat.reshape(P, FPER),
                        "w_i": _W["i"], "w_im": _W["im"],
                        "w_p2": _W["p2"], "w_m2": _W["m2"]})
    return in_maps


def kernel(points: np.ndarray):
    points = np.asarray(points)
    assert points.shape == (B, NPTS, 3) and points.dtype == np.float32

    nc = _get_nc()
    res = run_bass_kernel_spmd(nc, _make_in_maps(points),
                               core_ids=list(range(B)))

    out_points = np.empty((B, NPTS, 3), dtype=np.float32)
    coords = np.empty((B, NPTS, 3), dtype=np.int32)
    idxes = np.empty((B, NPTS), dtype=np.int32)
    offsets = np.empty((B, NPTS, 3), dtype=np.float32)
    valid = np.empty((B, NPTS), dtype=bool)
    for b in range(B):
        r = res.results[b]
        out_points[b] = r["o_pts"].reshape(-1)[: NPTS * 3].reshape(NPTS, 3)
        coords[b] = (r["o_coords"].view(np.uint16).astype(np.int32)
                     .reshape(-1)[: NPTS * 3].reshape(NPTS, 3)[:, ::-1] - 1)
        offsets[b] = r["o_off"].reshape(-1)[: NPTS * 3].reshape(NPTS, 3)
        idxes[b] = r["o_idx"].reshape(-1)[: NPTS] - 1
        valid[b] = r["o_valid"].reshape(-1)[: NPTS].astype(bool)
    return out_points, coords, idxes, offsets, valid


# revision 14
# speedup vs baseline: 1.5593x; 1.5593x over previous
"""DynamicVoxelizer Bass/Trainium2 kernel.

kernel(**inputs) takes FULL inputs (points: [8, 1_000_000, 3] f32), runs one
batch per NeuronCore (8 cores), returns FULL outputs matching reference():
  (out_points f32, coords_zyx i32, point_idxes i32, point_offsets f32,
   valid bool)

Device data layout is PLANAR: the host de-interleaves xyz so each SBUF
partition holds [x(7840) | y(7840) | z(7840)]. All per-component ops are then
unit-stride with scalar immediates, and the valid mask applies per-plane
without broadcast.

Numerics: XLA compiles the reference's (p-min)/0.2 into (p-min)*5 (verified
bit-exact on both backends), so q = RN(5*RN(p-min)).  floor(q) uses the
magic-number trick f0p1 = RN(q + 2^23) - (2^23 - 1) = round_nearest(q) + 1
(exact integer subtract), fixed to floor+1 with h = [q+1 < f0p1].
Outputs encode +1 offsets (coords+1, idx+1) so invalid lanes are exact 0s
from the mask multiply; the host subtracts 1.
"""

import numpy as np

import concourse.bass as bass
import concourse.mybir as mybir
import concourse.tile as tile
from concourse.bass_utils import run_bass_kernel_spmd

AOT = mybir.AluOpType
AFT = mybir.ActivationFunctionType
F32 = mybir.dt.float32
I32 = mybir.dt.int32
I16 = mybir.dt.int16
U8 = mybir.dt.uint8

B = 8                      # batch == number of cores
NPTS = 1_000_000           # points per batch
P = 128                    # SBUF partitions
NPER = 7840                # points per partition (padded): 128*7840 = 1_003_520
NPAD = P * NPER
FPER = NPER * 3            # floats per partition = 23520 (3 planes of 7840)
CH_N = 1120                # points per partition per chunk
CH_F = CH_N * 3
NCHUNKS = NPER // CH_N     # 7
PAD_VAL = 1.0e9            # pad points are far out of range -> invalid

MINS = (-51.2, -51.2, -3.0)
GRIDF = (512.0, 512.0, 30.0)
GPRIME = tuple(float(np.nextafter(np.float32(g), np.float32(0))) for g in GRIDF)
MAGIC = float(np.float32(2.0 ** 23))
MAGIC1 = float(np.float32(2.0 ** 23 - 1.0))

_cached = {}


def _split_excess_waits(nc, limit=1):
    """walrus here rejects instructions with more than `limit` sync-wait
    conditions; split the excess into standalone event-sem waits."""
    for fn in nc.m.functions:
        for bb in fn.blocks:
            il = bb.instructions
            new = []
            for inst in il:
                si = getattr(inst, "sync_info", None)
                if si is not None and si.on_wait and len(si.on_wait) > limit:
                    waits = list(si.on_wait)
                    excess, keep = waits[:-limit], waits[-limit:]
                    for j, w in enumerate(excess):
                        ev = mybir.InstEventSemaphore(
                            name=f"{inst.name}-xw{j}", ins=[], outs=[],
                            sync_info=mybir.SyncInfo(on_wait=[w], on_update=[]),
                        )
                        ev.engine = inst.engine
                        new.append(ev)
                    inst.sync_info = mybir.SyncInfo(
                        on_wait=keep, on_update=list(si.on_update))
                new.append(inst)
            il[:] = new


def _build_kernel():
    nc = bass.Bass()
    for v in sorted({-g for g in GPRIME}):
        t = nc.alloc_sbuf_tensor(f"const-f32-{v}", [P, 1], F32)
        nc.gpsimd.memset(t.ap(), v)
        nc.const_aps.aps[(F32, v)] = t.ap()
    nc.all_engine_barrier()

    pts = nc.dram_tensor("pts", [P, FPER], F32, kind="ExternalInput")
    o_pts = nc.dram_tensor("o_pts", [P, FPER], F32, kind="ExternalOutput")
    o_coords = nc.dram_tensor("o_coords", [P, FPER], I16, kind="ExternalOutput")
    o_off = nc.dram_tensor("o_off", [P, FPER], F32, kind="ExternalOutput")
    o_idx = nc.dram_tensor("o_idx", [P, NPER], I32, kind="ExternalOutput")
    o_valid = nc.dram_tensor("o_valid", [P, NPER], U8, kind="ExternalOutput")

    # DRAM chunk view: per plane c, columns [c*NPER + ch*CH_N, +CH_N)
    def dram_chunk(t, ch):
        v = t[:].rearrange("p (c n) -> p c n", c=3)
        return v[:, :, ch * CH_N:(ch + 1) * CH_N]

    with tile.TileContext(nc) as tc:
        with tc.tile_pool(name="io", bufs=2) as iop, \
             tc.tile_pool(name="scr", bufs=1) as scr:
            for ch in range(NCHUNKS):
                n0_ = ch * CH_N

                pt = iop.tile([P, CH_F], F32, tag="pts")
                nc.sync.dma_start(
                    pt[:].rearrange("p (c n) -> p c n", c=3),
                    dram_chunk(pts, ch))

                def pl(tile_, c):
                    return tile_[:, c * CH_N:(c + 1) * CH_N]

                # t3 = p - min  [ACT per plane]
                t3 = scr.tile([P, CH_F], F32, tag="t3")
                for c in range(3):
                    nc.scalar.activation(pl(t3, c), pl(pt, c),
                                         AFT.Copy, bias=-MINS[c], scale=1.0)
                # q0 = RN(5*t3)  [ACT, full width]
                q0 = scr.tile([P, CH_F], F32, tag="q0")
                nc.scalar.activation(q0[:], t3[:], AFT.Copy, bias=0.0, scale=5.0)

                # f0p1 = round_nearest(q0) + 1  [DVE ts 2x, magic number]
                f0p1 = scr.tile([P, CH_F], F32, tag="f0p1")
                nc.vector.tensor_scalar(f0p1[:], q0[:], MAGIC, MAGIC1,
                                        AOT.add, AOT.subtract)
                # h = [q0 + 1 < f0p1]; cfp1 = f0p1 - h = floor(q0)+1 (in place)
                h = scr.tile([P, CH_F], F32, tag="h")
                nc.vector.scalar_tensor_tensor(
                    h[:], q0[:], 1.0, f0p1[:], AOT.add, AOT.is_lt)
                cfp1 = f0p1
                nc.vector.scalar_tensor_tensor(
                    cfp1[:], h[:], -1.0, f0p1[:], AOT.mult, AOT.add)

                # validity: w = Abs(2*q0 - g'); sg = Sign(w - g')  [ACT]
                w3 = scr.tile([P, CH_F], F32, tag="w3")
                for c in range(3):
                    nc.scalar.activation(pl(w3, c), pl(q0, c),
                                         AFT.Abs, bias=-GPRIME[c], scale=2.0)
                for c in range(3):
                    nc.scalar.activation(pl(w3, c), pl(w3, c),
                                         AFT.Sign, bias=-GPRIME[c], scale=1.0)
                v01 = scr.tile([P, CH_N], F32, tag="v01")
                nc.vector.tensor_tensor(v01[:], pl(w3, 0), pl(w3, 1), AOT.max)
                mx = scr.tile([P, CH_N], F32, tag="mx")
                nc.vector.tensor_tensor(mx[:], v01[:], pl(w3, 2), AOT.max)
                valid = iop.tile([P, CH_N], U8, tag="valid")
                nc.vector.tensor_scalar(valid[:], mx[:], 0.0, None, AOT.is_le)

                # coords+1 = cfp1 * valid  -> i16 planar  [DVE per plane]
                cmem = iop.tile([P, CH_F], I16, tag="cmem")
                for c in range(3):
                    nc.vector.tensor_tensor(pl(cmem, c), pl(cfp1, c),
                                            valid[:], AOT.mult)

                # offsets: off0 = t3 - 0.2*cfp1 (into t3); then
                # off = (off0 + 0.1) * valid
                off0 = t3
                for c in range(3):
                    nc.vector.scalar_tensor_tensor(
                        pl(off0, c), pl(cfp1, c), -0.2, pl(t3, c),
                        AOT.mult, AOT.add)
                offm = iop.tile([P, CH_F], F32, tag="offm")
                for c in range(3):
                    nc.vector.scalar_tensor_tensor(
                        pl(offm, c), pl(off0, c), 0.1, valid[:],
                        AOT.add, AOT.mult)

                # out_points = p * valid  [GP per plane, in place]
                for c in range(3):
                    nc.gpsimd.tensor_tensor(pl(pt, c), pl(pt, c),
                                            valid[:], AOT.mult)

                # idx+1 = (iota+1) * valid  [GP iota + DVE tt]
                iot = scr.tile([P, CH_N], I32, tag="iot")
                nc.gpsimd.iota(iot[:], [[1, CH_N]], base=n0_ + 1,
                               channel_multiplier=NPER)
                imem = iop.tile([P, CH_N], I32, tag="imem")
                nc.vector.tensor_tensor(imem[:], iot[:], valid[:], AOT.mult)

                # stores
                nc.sync.dma_start(dram_chunk(o_pts, ch),
                                  pt[:].rearrange("p (c n) -> p c n", c=3))
                nc.sync.dma_start(dram_chunk(o_coords, ch),
                                  cmem[:].rearrange("p (c n) -> p c n", c=3))
                nc.sync.dma_start(dram_chunk(o_off, ch),
                                  offm[:].rearrange("p (c n) -> p c n", c=3))
                nc.sync.dma_start(o_idx[:, n0_:n0_ + CH_N], imem[:])
                nc.sync.dma_start(o_valid[:, n0_:n0_ + CH_N], valid[:])

    _split_excess_waits(nc, limit=1)
    return nc


def _get_nc():
    if "nc" not in _cached:
        _cached["nc"] = _build_kernel()
    return _cached["nc"]


def _make_in_maps(points):
    in_maps = []
    for b in range(B):
        arr = np.full((NPAD, 3), PAD_VAL, dtype=np.float32)
        arr[:NPTS] = points[b]
        # planar: [P, 3, NPER] with per-partition planes x|y|z
        planar = np.ascontiguousarray(
            arr.reshape(P, NPER, 3).transpose(0, 2, 1)).reshape(P, FPER)
        in_maps.append({"pts": planar})
    return in_maps


def _unplanar(a):
    # [P, FPER]-planar -> [NPAD, 3] xyz rows
    return np.ascontiguousarray(
        a.reshape(P, 3, NPER).transpose(0, 2, 1)).reshape(NPAD, 3)


def kernel(points: np.ndarray):
    points = np.asarray(points)
    assert points.shape == (B, NPTS, 3) and points.dtype == np.float32

    nc = _get_nc()
    res = run_bass_kernel_spmd(nc, _make_in_maps(points),
                               core_ids=list(range(B)))

    out_points = np.empty((B, NPTS, 3), dtype=np.float32)
    coords = np.empty((B, NPTS, 3), dtype=np.int32)
    idxes = np.empty((B, NPTS), dtype=np.int32)
    offsets = np.empty((B, NPTS, 3), dtype=np.float32)
    valid = np.empty((B, NPTS), dtype=bool)
    for b in range(B):
        r = res.results[b]
        out_points[b] = _unplanar(r["o_pts"])[:NPTS]
        coords[b] = (_unplanar(r["o_coords"]).astype(np.int32)
                     [:NPTS, ::-1] - 1)
        offsets[b] = _unplanar(r["o_off"])[:NPTS]
        idxes[b] = r["o_idx"].reshape(-1)[:NPTS] - 1
        valid[b] = r["o_valid"].reshape(-1)[:NPTS].astype(bool)
    return out_points, coords, idxes, offsets, valid
